# revision 2
# baseline (speedup 1.0000x reference)
"""CKAFormer distributed Bass kernel for 8 TRN2 NeuronCores — v2.

Reference computation (DEPTH=4 iterations on X [32768, 512]):
    X = X / ||X||_row
    P = softmax(relu(X@W1+b1)@W2+b2)          # [N, 64]
    X = X + g*(P @ (P.T @ X))
    C = X.T @ X   (global)
    X = X - g*(X @ C)
  out = relu(X@W1+b1)@W2+b2                   # [N, 64]

v2 structure per iteration (X row-sharded, 4096 tokens/core):
  p1: normalize (V: fp8 Xn8=8*Xn, GP: bf16 Xn) + PE transposes Xn8->XnT8
      + A = Xn8^T Xn8 in fp8 DoubleRow interleaved per token-tile-pair.
      One fp8 AllReduce of A/4 at p1 end (overlapped by all of p2).
  p2: MLP mm1 (fp8 DR) -> relu -> logits -> exp -> softmax (P8=16*P fp8)
      -> B = P8^T Xn8 fp8 DR -> bf16 AllReduce(B) -> PT transposes.
  p3: per tile one PSUM chain: XnT8-DR x2 (-2048g*Xn@A) + PT@Bg (2048g*P@B);
      Y = psx/2048 + Xn via one vector scalar_tensor_tensor; scalar Square
      accumulates next iteration's row norms from Y.
Final MLP streamed per 4-tile group (transpose -> bf16 mm1 -> logits -> DMA).
Algebra: C ~= A (2g B^T B and g^2 terms dropped, <1e-6 rel), X@C uses
pre-P@B X (g^2 term dropped) — same approximations as validated baseline.
"""

import numpy as np

import concourse.bass as bass
import concourse.mybir as mybir
import concourse.tile as tile
from concourse import bacc
from concourse.bass import ts
from concourse.bass_utils import run_bass_kernel_spmd
from concourse.masks import make_identity

AF = mybir.ActivationFunctionType
ALU = mybir.AluOpType
FP32 = mybir.dt.float32
BF16 = mybir.dt.bfloat16
FP8 = mybir.dt.float8e4
DR = mybir.MatmulPerfMode.DoubleRow

N_CORES = 8
N_TOK = 32768
NS = N_TOK // N_CORES  # 4096 tokens per core
D = 512
HID = 16
OUT = 64
DEPTH = 4
GAMMA = 1e-4
NT = NS // 128  # 32 token tiles
DC = D // 128  # 4 feature chunks

_NC_CACHE = None


def _copy(nc, idx, out, in_):
    if idx % 2 == 0:
        nc.scalar.activation(out, in_, AF.Copy)
    else:
        nc.vector.tensor_copy(out, in_)


def _build_body(nc, tc, X, W1, b1, W2, b2, out):
    import contextlib

    cm = contextlib.ExitStack()
    with cm:
        mp = cm.enter_context(tc.tile_pool(name="mp", bufs=1))
        scr = cm.enter_context(tc.tile_pool(name="scr", bufs=2))
        ps = cm.enter_context(tc.tile_pool(name="ps", bufs=1, space="PSUM"))
        dp = cm.enter_context(tc.tile_pool(name="dp", bufs=1, space="DRAM"))

        # ---- X shard load: issue FIRST (16 pieces; compute streams behind) --
        stage_cm = tc.tile_pool(name="stagep", bufs=1)
        stagep = stage_cm.__enter__()
        stage = stagep.tile([128, NT * D], FP32, tag="stage")
        stage_v = stage[:].rearrange("p (t d) -> p t d", t=NT)
        x_v = X.rearrange("(t p) d -> p t d", p=128)
        for i in range(16):
            nc.sync.dma_start(stage_v[:, ts(i, NT // 16), :], x_v[:, ts(i, NT // 16), :])

        # ---- constants -----------------------------------------------------
        idn = mp.tile([128, 128], BF16, tag="idn")
        make_identity(nc, idn)

        w1f = mp.tile([128, DC * HID], FP32, tag="w1f")
        nc.sync.dma_start(
            w1f[:].rearrange("p (c h) -> p c h", c=DC),
            W1.rearrange("(c p) h -> p c h", p=128),
        )
        w1sb = mp.tile([128, DC * HID], BF16, tag="w1sb")
        nc.vector.tensor_copy(w1sb[:], w1f[:])
        w1sb8 = mp.tile([128, DC * HID], FP8, tag="w1sb8")
        nc.vector.tensor_scalar_mul(w1sb8[:], w1f[:], 16.0)  # fp8(16*W1)

        b1t = mp.tile([HID, 1], FP32, tag="b1t")
        nc.sync.dma_start(b1t[:], b1.unsqueeze(1))
        w2f = mp.tile([HID + 1, OUT], FP32, tag="w2f")
        nc.sync.dma_start(w2f[0:HID, :], W2)
        nc.sync.dma_start(w2f[HID : HID + 1, :], b2.unsqueeze(0))
        w2p = mp.tile([HID + 1, OUT], BF16, tag="w2p")
        nc.vector.tensor_copy(w2p[:], w2f[:])

        # ---- persistent state ----------------------------------------------
        Xn = mp.tile([128, NT * D], BF16, tag="Xn")
        Xn8 = mp.tile([128, NT * D], FP8, tag="Xn8")  # fp8(8*Xn)
        XnT8 = mp.tile([128, DC * NS], FP8, tag="XnT8")  # fp8(8*Xn^T)
        Y = mp.tile([128, NT * D], BF16, tag="Y")
        Pb = mp.tile([128, NT * OUT], BF16, tag="Pb")
        Eb = mp.tile([128, NT * OUT], BF16, tag="Eb")
        PT = mp.tile([OUT, NS], BF16, tag="PT")  # P^T
        Hp = mp.tile([HID + 1, NS], BF16, tag="Hp")
        nc.vector.memset(Hp[:], 1.0)  # ones row HID for the b2 trick

        xn8_v = Xn8[:].rearrange("p (t d) -> p t d", t=NT)
        xnt_v = XnT8[:].rearrange("p (c n) -> p c n", c=DC)
        w18_v = w1sb8[:].rearrange("p (c h) -> p c h", c=DC)

        # ---- warmup collectives (absorb ncfw first-call latency) -----------
        wu_sb = mp.tile([1, 128], FP8, tag="wu_sb")
        nc.vector.memset(wu_sb[:], 0.0)
        wu_in = dp.tile([1, 128], FP8, tag="wu_in")
        wu_out = dp.tile([1, 128], FP8, tag="wu_out")
        nc.sync.dma_start(wu_in[:], wu_sb[:])
        nc.gpsimd.collective_compute(
            "AllReduce", ALU.add, replica_groups=[list(range(N_CORES))],
            ins=[wu_in.opt()], outs=[wu_out.opt()],
        )
        wu_sb2 = mp.tile([1, 128], BF16, tag="wu_sb2")
        nc.vector.memset(wu_sb2[:], 0.0)
        wu_in2 = dp.tile([1, 128], BF16, tag="wu_in2")
        wu_out2 = dp.tile([1, 128], BF16, tag="wu_out2")
        nc.sync.dma_start(wu_in2[:], wu_sb2[:])
        nc.gpsimd.collective_compute(
            "AllReduce", ALU.add, replica_groups=[list(range(N_CORES))],
            ins=[wu_in2.opt()], outs=[wu_out2.opt()],
        )

        # ---- prologue: iter-0 row sums of squares (streams behind DMA) -----
        ssq = scr.tile([128, NT], FP32, tag="ssq")
        for t in range(NT):
            sqs = scr.tile([128, D], BF16, tag="sqs", bufs=2)
            if t % 2 == 0:
                nc.scalar.activation(
                    sqs[:], stage[:, ts(t, D)], AF.Square,
                    accum_out=ssq[:, t : t + 1],
                )
            else:
                nc.vector.scalar_tensor_tensor(
                    sqs[:], stage[:, ts(t, D)], 1.0, stage[:, ts(t, D)],
                    ALU.mult, ALU.mult, accum_out=ssq[:, t : t + 1],
                )

        for it in range(DEPTH):
            src = stage if it == 0 else Y
            last = it == DEPTH - 1

            # ---- p1: normalize + transpose + A (fp8 DR, interleaved) -------
            s_norm = scr.tile([128, NT], FP32, tag="s_norm")
            inv_s = scr.tile([128, NT], FP32, tag="inv_s")
            psA = [
                ps.tile([128, D], FP32, tag=f"psA{c}", name=f"psA{c}_{it}")
                for c in range(DC)
            ]
            for g in range(NT // 8):
                nc.scalar.activation(
                    s_norm[:, ts(g, 8)], ssq[:, ts(g, 8)], AF.Sqrt
                )
                nc.vector.reciprocal(inv_s[:, ts(g, 8)], s_norm[:, ts(g, 8)])
                for t in range(8 * g, 8 * g + 8):
                    nc.vector.tensor_scalar_mul(
                        Xn[:, ts(t, D)], src[:, ts(t, D)], inv_s[:, t : t + 1]
                    )
                    if t % 2 == 0:
                        nc.scalar.activation(
                            Xn8[:, ts(t, D)], Xn[:, ts(t, D)], AF.Copy, scale=8.0
                        )
                    else:
                        nc.vector.tensor_scalar_mul(
                            Xn8[:, ts(t, D)], Xn[:, ts(t, D)], 8.0
                        )
                    pst = ps.tile([128, D], BF16, tag="psT", bufs=2)
                    for dc in range(DC):
                        nc.tensor.transpose(
                            pst[:, ts(dc, 128)],
                            Xn[:, t * D + dc * 128 : t * D + (dc + 1) * 128],
                            idn[:],
                        )
                    if t % 2 == 0:
                        nc.scalar.activation(
                            xnt_v[:, :, ts(t, 128)],
                            pst[:].rearrange("p (c n) -> p c n", c=DC),
                            AF.Copy, scale=8.0,
                        )
                    else:
                        nc.vector.tensor_scalar_mul(
                            xnt_v[:, :, ts(t, 128)],
                            pst[:].rearrange("p (c n) -> p c n", c=DC),
                            8.0,
                        )
                    if t % 2 == 1:
                        k = t // 2
                        for c in range(DC):
                            nc.tensor.matmul(
                                psA[c][:],
                                xn8_v[:, 2 * k : 2 * k + 2, ts(c, 128)],
                                xn8_v[:, 2 * k : 2 * k + 2, :],
                                start=(k == 0),
                                stop=(k == NT // 2 - 1),
                                perf_mode=DR,
                            )

            if it == 0:
                stage_cm.__exit__(None, None, None)

            # ---- A out (fp8, A/4) + single fp8 AllReduce -------------------
            aloc = scr.tile([128, DC * D], FP8, tag="aloc")
            arA_in = dp.tile([D, D], FP8, tag="arA_in", bufs=2)
            arA_out = dp.tile([D, D], FP8, tag="arA_out", bufs=2)
            aloc_v = aloc[:].rearrange("p (c n) -> p c n", c=DC)
            arA_iv = arA_in[:].rearrange("(c p) n -> p c n", p=128)
            for c in range(DC):
                if c % 2 == 0:
                    nc.vector.tensor_scalar_mul(
                        aloc[:, ts(c, D)], psA[c][:], 1.0 / 256.0
                    )
                else:
                    nc.scalar.activation(
                        aloc[:, ts(c, D)], psA[c][:], AF.Copy, scale=1.0 / 256.0
                    )
                nc.sync.dma_start(arA_iv[:, c : c + 1, :], aloc_v[:, c : c + 1, :])
            nc.gpsimd.collective_compute(
                "AllReduce", ALU.add, replica_groups=[list(range(N_CORES))],
                ins=[arA_in.opt()], outs=[arA_out.opt()],
            )

            # ---- p2: MLP -> logits -> softmax -> B -> PT -------------------
            for c8 in range(NS // 512):
                psh = ps.tile([HID, 512], FP32, tag="psH")
                for j in range(DC // 2):
                    nc.tensor.matmul(
                        psh[:],
                        w18_v[:, 2 * j : 2 * j + 2, :],
                        xnt_v[:, 2 * j : 2 * j + 2, ts(c8, 512)],
                        start=(j == 0),
                        stop=(j == DC // 2 - 1),
                        perf_mode=DR,
                    )
                nc.scalar.activation(
                    Hp[0:HID, ts(c8, 512)], psh[:], AF.Relu, bias=b1t[:],
                    scale=1.0 / 128.0,
                )

            sums = scr.tile([128, NT], FP32, tag="sums")
            rsum = scr.tile([128, NT], FP32, tag="rsum")
            psb = ps.tile([128, D], FP32, tag="psA1", name=f"psb_{it}")
            for g in range(NT // 8):
                for t in range(8 * g, 8 * g + 8):
                    pslt = ps.tile(
                        [128, D], FP32, tag=f"psA{[0, 2, 3][t % 3]}", name=f"psl_{it}_{t}"
                    )
                    psl = pslt[:, 0:OUT]
                    nc.tensor.matmul(
                        psl, Hp[:, ts(t, 128)], w2p[:], start=True, stop=True
                    )
                    nc.scalar.activation(Eb[:, ts(t, OUT)], psl, AF.Exp)
                    nc.vector.tensor_reduce(
                        sums[:, t : t + 1],
                        Eb[:, ts(t, OUT)],
                        mybir.AxisListType.X,
                        ALU.add,
                    )
                nc.vector.reciprocal(rsum[:, ts(g, 8)], sums[:, ts(g, 8)])
                for t in range(8 * g, 8 * g + 8):
                    nc.vector.tensor_scalar_mul(
                        Pb[:, ts(t, OUT)], Eb[:, ts(t, OUT)], rsum[:, t : t + 1]
                    )
                    nc.tensor.matmul(
                        psb[0:OUT, :],
                        Pb[:, ts(t, OUT)],
                        Xn[:, ts(t, D)],
                        start=(t == 0),
                        stop=(t == NT - 1),
                    )

            # B AllReduce (bf16) — queued on CC behind the A AllReduce
            bloc = scr.tile([OUT, D], BF16, tag="bloc")
            nc.vector.tensor_copy(bloc[:], psb[0:OUT, :])
            b_in = dp.tile([OUT, D], BF16, tag="b_in", bufs=2)
            b_out = dp.tile([OUT, D], BF16, tag="b_out", bufs=2)
            nc.sync.dma_start(b_in[:], bloc[:])
            nc.gpsimd.collective_compute(
                "AllReduce", ALU.add, replica_groups=[list(range(N_CORES))],
                ins=[b_in.opt()], outs=[b_out.opt()],
            )

            # PT transposes (fills the AllReduce window)
            for q in range(NT // 4):
                psp = ps.tile([128, D], BF16, tag="psp", bufs=1, name=f"psp_{it}_{q}")
                for j in range(4):
                    nc.tensor.transpose(
                        psp[0:OUT, ts(j, 128)], Pb[:, ts(4 * q + j, OUT)], idn[:]
                    )
                nc.vector.tensor_copy(PT[:, ts(q, 512)], psp[0:OUT, :])

            # Cneg = fp8(-256g * A_global) from the fp8 AR (A_global/4)
            Asb = scr.tile([128, DC * D], FP8, tag="Asb")
            asb_v = Asb[:].rearrange("p (c n) -> p c n", c=DC)
            arA_ov = arA_out[:].rearrange("(c p) n -> p c n", p=128)
            nc.sync.dma_start(asb_v[:, 0:2, :], arA_ov[:, 0:2, :])
            nc.sync.dma_start(asb_v[:, 2:4, :], arA_ov[:, 2:4, :])
            Cneg = scr.tile([128, DC * D], FP8, tag="Cneg")
            nc.vector.tensor_scalar_mul(
                Cneg[:, 0 : 2 * D], Asb[:, 0 : 2 * D], -1024.0 * GAMMA
            )
            nc.vector.tensor_scalar_mul(
                Cneg[:, 2 * D : 4 * D], Asb[:, 2 * D : 4 * D], -1024.0 * GAMMA
            )
            cneg_v = Cneg[:].rearrange("p (c n) -> p c n", c=DC)

            # Bg = 2048g * B_global
            Bsb = scr.tile([OUT, D], BF16, tag="Bsb")
            nc.sync.dma_start(Bsb[:], b_out[:])
            Bg = scr.tile([OUT, D], BF16, tag="Bg")
            nc.vector.tensor_scalar_mul(Bg[:], Bsb[:], 2048.0 * GAMMA)

            # ---- p3: chain  psx = -2048g*Xn@A + 2048g*P@B ------------------
            if not last:
                ssq_next = scr.tile([128, NT], FP32, tag="ssq")
            for t in range(NT):
                psx = ps.tile([128, D], FP32, tag=f"psA{t % 4}", name=f"psx_{it}_{t}")
                for j in range(DC // 2):
                    nc.tensor.matmul(
                        psx[:],
                        xnt_v[:, 2 * j : 2 * j + 2, ts(t, 128)],
                        cneg_v[:, 2 * j : 2 * j + 2, :],
                        start=(j == 0),
                        stop=False,
                        perf_mode=DR,
                    )
                nc.tensor.matmul(
                    psx[:], PT[:, ts(t, 128)], Bg[:], start=False, stop=True
                )
                nc.vector.scalar_tensor_tensor(
                    Y[:, ts(t, D)], psx[:], 1.0 / 2048.0, Xn[:, ts(t, D)],
                    ALU.mult, ALU.add,
                )
                if not last:
                    sqs = scr.tile([128, D], BF16, tag="sqs", bufs=2)
                    nc.scalar.activation(
                        sqs[:], Y[:, ts(t, D)], AF.Square,
                        accum_out=ssq_next[:, t : t + 1],
                    )
            if not last:
                ssq = ssq_next

        # ---- final MLP on un-normalized Y, streamed per 4-tile group -------
        fp = cm.enter_context(tc.tile_pool(name="fp", bufs=1))
        slab_all = fp.tile([128, 2 * DC * 512], BF16, tag="slab")
        for q in range(NT // 4):
            slab = slab_all[:, (q % 2) * DC * 512 : (q % 2 + 1) * DC * 512]
            slab_v = slab.rearrange("p (c n) -> p c n", c=DC)
            for j in range(4):
                t = 4 * q + j
                pst = ps.tile([128, D], BF16, tag="psT", bufs=2, name=f"psf_{t}")
                for dc in range(DC):
                    nc.tensor.transpose(
                        pst[:, ts(dc, 128)],
                        Y[:, t * D + dc * 128 : t * D + (dc + 1) * 128],
                        idn[:],
                    )
                _copy(
                    nc, t,
                    slab_v[:, :, ts(j, 128)],
                    pst[:].rearrange("p (c n) -> p c n", c=DC),
                )
            psh = ps.tile([HID, 512], FP32, tag="psH", name=f"pshf_{q}")
            for kc in range(DC):
                nc.tensor.matmul(
                    psh[:],
                    w1sb[:, ts(kc, HID)],
                    slab_v[:, kc, :],
                    start=(kc == 0),
                    stop=(kc == DC - 1),
                )
            hp_loc = Hp[:, ts(q % 2, 512)]
            nc.scalar.activation(hp_loc[0:HID, :], psh[:], AF.Relu, bias=b1t[:])
            osb = fp.tile([128, 4 * OUT], FP32, tag="osb", bufs=2)
            for j in range(4):
                pslt = ps.tile(
                    [128, D], FP32, tag=f"psA{j % 2}", name=f"pslf_{q}_{j}"
                )
                psl = pslt[:, 0:OUT]
                nc.tensor.matmul(
                    psl, hp_loc[:, ts(j, 128)], w2p[:], start=True, stop=True
                )
                _copy(nc, j, osb[:, ts(j, OUT)], psl)
            out_v = out.rearrange("(q j p) o -> p q j o", p=128, j=4)
            nc.sync.dma_start(
                out_v[:, q, :, :],
                osb[:].rearrange("p (j o) -> p j o", j=4),
            )


def build_nc():
    global _NC_CACHE
    if _NC_CACHE is not None:
        return _NC_CACHE
    nc = bacc.Bacc("TRN2", debug=False, num_devices=N_CORES)
    X = nc.dram_tensor("X", [NS, D], FP32, kind="ExternalInput").ap()
    W1 = nc.dram_tensor("W1", [D, HID], FP32, kind="ExternalInput").ap()
    b1 = nc.dram_tensor("b1", [HID], FP32, kind="ExternalInput").ap()
    W2 = nc.dram_tensor("W2", [HID, OUT], FP32, kind="ExternalInput").ap()
    b2 = nc.dram_tensor("b2", [OUT], FP32, kind="ExternalInput").ap()
    out = nc.dram_tensor("out", [NS, OUT], FP32, kind="ExternalOutput").ap()
    with tile.TileContext(nc) as tc:
        _build_body(nc, tc, X, W1, b1, W2, b2, out)
    nc.compile()
    _NC_CACHE = nc
    return nc


def run(inputs, trace=False):
    X = np.ascontiguousarray(np.asarray(inputs["X"], dtype=np.float32))
    W1 = np.ascontiguousarray(np.asarray(inputs["W1"], dtype=np.float32))
    b1 = np.ascontiguousarray(np.asarray(inputs["b1"], dtype=np.float32))
    W2 = np.ascontiguousarray(np.asarray(inputs["W2"], dtype=np.float32))
    b2 = np.ascontiguousarray(np.asarray(inputs["b2"], dtype=np.float32))
    nc = build_nc()
    in_maps = [
        {"X": X[i * NS : (i + 1) * NS], "W1": W1, "b1": b1, "W2": W2, "b2": b2}
        for i in range(N_CORES)
    ]
    res = run_bass_kernel_spmd(nc, in_maps, core_ids=list(range(N_CORES)), trace=trace)
    full = np.concatenate([r["out"] for r in res.results], axis=0)
    return full, res


def kernel(**inputs):
    full, _ = run(inputs, trace=False)
    return full
